# revision 5
# baseline (speedup 1.0000x reference)
"""Trainium2 Bass kernel for nn_BagKQMClassModel.

Computation (per batch item b):
    K[b,n,m]   = exp(-d2/(2 s^2)),  d2 = |A[b,n] - C[m]|^2
    out_w[b,m] = (1/N) sum_n comp_w[m] * K^2
    y_w        = out_w / sum_m out_w
    probs      = y_w @ (y_v^2),  y_v = c_y rows normalized

Key transformations:
  * K^2 = exp(-d2/s^2): one exp per (b,n,m) element.
  * d2 folded into one fp8e4 DoubleRow matmul (2x PE rate vs 16-bit).
    Contraction is 36 rows = 2 k-tiles of 18:
        rows 0-31: data a_d * c_d (both e4m3-quantized)
        row 32: CT 1,     AT -a2/2 (e4m3)    row 33: CT 1,     AT resid
        row 34: CT -b2/2, AT 1               row 35: CT resid, AT 1
    The residual rows carry the e4m3 quantization error of the -a2/2 and
    -b2/2 terms (error feedback), keeping final rel err ~1e-4.
  * probs = T[:, :10] / T[:, 10], T = sum_{n,m} K2 * W with
    W[m, :10] = comp_w[m] * c_y[m]^2 / |c_y[m]|^2, W[m, 10] = comp_w[m].
  * m and (b,n) are PERMUTED vs the reference layout so every input DMA
    is contiguous (partition p holds a contiguous DRAM block):
    m = 16p + r,  bn: p = 4t + n//32, j = n%32.  All downstream sums are
    permutation-invariant since c_y/comp_w use the same m-permutation.
  * One PSUM tile S[11, 128] accumulates mm2 over all 64 steps; final
    per-batch output is a single [11, 32, 4] free-dim reduce + transpose.
  * exp runs on 2048-wide ACT windows (2 m-chunks) to amortize the ~300
    cycle ACT instruction overhead.  PSUM: 3 x 2-bank g ring + 1 bank S
    + 1 bank transpose scratch = 8 banks.
  * The 8:1 bag reduction uses one DVE op per halving level across the
    whole window (amortizes the ~151-cycle DVE op overhead).
  * CT chunks 2-15 and AT blocks 1-3 are prepared DURING the main loop
    in PE/DVE slack; transposes process two 36-row groups per
    instruction ([128, 72] -> [72, 128]) to halve PE transpose count.

Sharding: batch 256 -> 32 items per core across 8 cores; c_x/c_y/comp_w
replicated. No collectives (forward only).
"""

import numpy as np

import concourse.bacc as bacc
import concourse.mybir as mybir
import concourse.tile as tile
from concourse.bass_utils import run_bass_kernel_spmd
from concourse.masks import make_identity

NCORES = 8
BS, N, DX, DY, M = 256, 128, 32, 10, 2048
BPC = BS // NCORES      # 32 batch items per core
MB = M // 128           # 16 chunks of the component axis
KT = 18                 # k-tile size for fp8 DoubleRow (2 x 18 = 36 rows)
KAUG = 2 * KT           # 36 = 32 data + 2 aug + 2 aug
NJ = 32                 # bn = 32 j-slices x 128 p
JB = 8                  # j slices per block
NBLK = NJ // JB         # 4 blocks
F_CHUNK = JB * 128      # 1024 bn columns per (m-chunk, block) step
MIN_SIGMA = 1e-3
FP32 = mybir.dt.float32
FP16 = mybir.dt.float16
FP8 = mybir.dt.float8e4
AX = mybir.AxisListType
ALU = mybir.AluOpType
ACTF = mybir.ActivationFunctionType
DR = mybir.MatmulPerfMode.DoubleRow


def _body(tc, inp, cx, cy, cw_d, out_d, scale):
    nc = tc.nc
    from contextlib import ExitStack

    with ExitStack() as ctx:
        const = ctx.enter_context(tc.tile_pool(name="const", bufs=1))
        work = ctx.enter_context(tc.tile_pool(name="work", bufs=2))
        k2p = ctx.enter_context(tc.tile_pool(name="k2p", bufs=3))
        psum = ctx.enter_context(tc.tile_pool(name="psum", bufs=1, space="PSUM"))

        # ---- contiguous input DMAs ----------------------------------------
        A_nat = const.tile([128, NJ, DX], FP32)     # p = 4t + n//32, j = n%32
        inp_r = inp.rearrange("t (a j) d -> (t a) j d", a=4)
        nc.sync.dma_start(out=A_nat[:, 0:16, :], in_=inp_r[:, 0:16, :])
        nc.scalar.dma_start(out=A_nat[:, 16:32, :], in_=inp_r[:, 16:32, :])
        cx_nat = const.tile([128, MB, DX], FP32)    # m = 16p + r
        nc.sync.dma_start(out=cx_nat, in_=cx.rearrange("(p r) d -> p r d", p=128))
        cy_nat = const.tile([128, MB, DY], FP32)
        nc.gpsimd.dma_start(out=cy_nat, in_=cy.rearrange("(p r) d -> p r d", p=128))
        cw_nat = const.tile([128, MB], FP32)
        nc.gpsimd.dma_start(out=cw_nat, in_=cw_d.rearrange("(p r) -> p r", p=128))

        ident8 = const.tile([128, 128], FP8)
        make_identity(nc, ident8)
        ident32 = const.tile([DY + 1, DY + 1], FP32)
        make_identity(nc, ident32)

        # packed fp8 transpose sources
        cx_pack = const.tile([128, MB, KAUG], FP8)  # [d x32, 1, 1, b2_8, b2_res]
        A_pack = const.tile([128, NJ, KAUG], FP8)   # [d x32, a2_8, a2_res, 1, 1]
        nc.gpsimd.memset(cx_pack[:, :, DX : DX + 2], 1.0)
        nc.gpsimd.memset(A_pack[:, :, DX + 2 : DX + 4], 1.0)

        # DoubleRow k-tile operand layouts
        CT8 = const.tile([KT, 2, MB, 128], FP8)
        AT8 = const.tile([KT, 2, NJ, 128], FP8)

        W16 = const.tile([128, MB, DY + 1], FP16)
        S = psum.tile([DY + 1, 128], FP32, tag="S")

        one3 = lambda t: t.rearrange("p (s o) -> p s o", o=1)

        # ---- prep helpers --------------------------------------------------
        def quant_chain(nat, pack, lo, hi, col8):
            """data cast + e4m3(-|x|^2/2) + residual into pack[:, lo:hi]."""
            span = hi - lo
            sq = work.tile([128, span, DX], FP32, tag="sq", bufs=2)
            nc.vector.tensor_mul(sq, nat[:, lo:hi, :], nat[:, lo:hi, :])
            mh = work.tile([128, span], FP32, tag="mh", bufs=2)
            nc.vector.tensor_reduce(out=one3(mh), in_=sq, axis=AX.X, op=ALU.add)
            nc.vector.tensor_scalar_mul(mh, mh, -0.5)
            nc.vector.tensor_copy(pack[:, lo:hi, col8 : col8 + 1], one3(mh))
            up = work.tile([128, span], FP32, tag="up", bufs=2)
            nc.vector.tensor_copy(one3(up), pack[:, lo:hi, col8 : col8 + 1])
            nc.vector.tensor_sub(
                pack[:, lo:hi, col8 + 1 : col8 + 2], one3(mh), one3(up)
            )
            nc.vector.tensor_copy(pack[:, lo:hi, 0:DX], nat[:, lo:hi, :])

        def transpose_pair(pack, dst8, idx, copy_fn):
            """Transpose pack[:, idx:idx+2, :] ([128, 72] fp8) and scatter the
            four 18-row groups into dst8[:, k, idx/idx+1, :]."""
            trk = psum.tile([2 * KAUG, 128, 2], FP8, tag="trk", bufs=1)
            nc.tensor.transpose(
                trk[:, :, 0],
                pack[:, idx : idx + 2, :].rearrange("p s d -> p (s d)"),
                ident8,
            )
            t72 = work.tile([2 * KAUG, 128], FP8, tag="t72", bufs=2)
            copy_fn(t72, trk[:, :, 0])
            for half in range(2):
                for k in range(2):
                    nc.gpsimd.dma_start(
                        out=dst8[:, k, idx + half, :],
                        in_=t72[half * KAUG + k * KT : half * KAUG + (k + 1) * KT, :],
                    )

        def w_chain():
            sqy = work.tile([128, MB, DY], FP32, tag="sqy")
            nc.vector.tensor_mul(sqy, cy_nat, cy_nat)
            ssum = work.tile([128, MB], FP32, tag="ssum")
            nc.vector.tensor_reduce(out=one3(ssum), in_=sqy, axis=AX.X, op=ALU.add)
            rec = work.tile([128, MB], FP32, tag="rec")
            nc.vector.reciprocal(rec, ssum)
            facr = work.tile([128, MB], FP32, tag="facr")
            nc.vector.tensor_mul(facr, rec, cw_nat)
            facr_b = one3(facr).broadcast_to([128, MB, DY])
            wtmp = work.tile([128, MB, DY], FP32, tag="wtmp")
            nc.vector.tensor_mul(wtmp, sqy, facr_b)
            nc.vector.tensor_copy(W16[:, :, 0:DY], wtmp)
            nc.vector.tensor_copy(W16[:, :, DY : DY + 1], one3(cw_nat))

        sc_copy = nc.scalar.copy
        ve_copy = nc.vector.tensor_copy

        # ---- prologue: cx, CT chunks 0-1, W, AT blocks 0-1 ----------------
        quant_chain(cx_nat, cx_pack, 0, MB, DX + 2)
        transpose_pair(cx_pack, CT8, 0, sc_copy)
        quant_chain(A_nat, A_pack, 0, JB, DX)
        for jp in range(4):                      # AT block 0
            transpose_pair(A_pack, AT8, 2 * jp, sc_copy)
        w_chain()
        quant_chain(A_nat, A_pack, JB, 2 * JB, DX)
        for jp in range(4, 8):                   # AT block 1
            transpose_pair(A_pack, AT8, 2 * jp, sc_copy)

        # deferred prep, scheduled by deadline (step index it must precede):
        # CT pair (c, c+1) before step c (first use, jb=0); AT block b
        # before step 16b.
        quanta = []
        for c in range(2, MB, 2):
            quanta.append(lambda c=c: transpose_pair(cx_pack, CT8, c, ve_copy))
        for blk in (2, 3):
            jlo = blk * JB
            quanta.append(lambda jlo=jlo: quant_chain(A_nat, A_pack, jlo, jlo + JB, DX))
            for jp in range(jlo // 2, jlo // 2 + 4):
                quanta.append(
                    lambda jp=jp: transpose_pair(A_pack, AT8, 2 * jp, ve_copy)
                )
        # 7 CT + 2*5 AT = 17 quanta; drain 1 per step: CT pair (2k, 2k+1)
        # emitted at step k-1 (first used at step 2k), block 2 done by step
        # 11 (used at 32), block 3 by step 16 (used at 48).
        sched = {s: [quanta[s]] if s < len(quanta) else [] for s in range(64)}

        AT8f = AT8.rearrange("p k j c -> p k (j c)")

        # ---- main loop ----------------------------------------------------
        # 64 steps (jb, c) of 1024 bn cols each; mm1 emits 256-col DoubleRow
        # pieces into a 2-window PSUM ring of 1536 fp32 each (6 banks); ACT
        # fires per filled window; the 8:1 bag reduce + mm2 batch 3 chunks
        # (= 2 windows = one contiguous 3072-col span of the 4-slot K2 ring).
        WIN = 1536
        K2r = const.tile([128, 4, WIN], FP16)
        K2f = K2r.rearrange("p w f -> p (w f)")
        TOT_STEPS = NBLK * MB
        mm2_emitted = [0]

        def emit_mm2_group(entry):
            r3t, steps = entry
            for i, s_ in enumerate(steps):
                nc.tensor.matmul(
                    S,
                    W16[:, s_ % MB, :],
                    r3t[:, i, :],
                    start=(s_ == 0),
                    stop=(s_ == TOT_STEPS - 1),
                )
                mm2_emitted[0] = s_ + 1

        def emit_reduce_group(grp):
            base = 3072 * (grp % 2)
            kv = K2f[:, base : base + 3072].rearrange("p (t f) -> p t f", t=3)
            r1 = work.tile([128, 3, 512], FP16, tag="r1")
            nc.vector.tensor_add(r1, kv[:, :, 0:512], kv[:, :, 512:1024])
            r2 = work.tile([128, 3, 256], FP16, tag="r2")
            nc.vector.tensor_add(r2, r1[:, :, 0:256], r1[:, :, 256:512])
            r3 = work.tile([128, 3, 128], FP16, tag="r3", bufs=3)
            nc.vector.tensor_add(r3, r2[:, :, 0:128], r2[:, :, 128:256])
            return (r3, [3 * grp, 3 * grp + 1, 3 * grp + 2])

        pieces = 0
        gw = None
        pending = []
        for s in range(TOT_STEPS):
            jb, c = divmod(s, MB)
            for q in range(4):
                if pieces % 6 == 0:
                    gw = psum.tile([128, WIN], FP32, tag="g", bufs=2, name="gw")
                off = (pieces * 256) % WIN
                nc.tensor.matmul(
                    gw[:, off : off + 256],
                    CT8[:, :, c, :],
                    AT8f[:, :, jb * F_CHUNK + q * 256 : jb * F_CHUNK + (q + 1) * 256],
                    start=True,
                    stop=True,
                    perf_mode=DR,
                )
                pieces += 1
                if pieces % 6 == 0:
                    w = pieces // 6 - 1
                    nc.scalar.activation(
                        K2r[:, w % 4, :], gw, ACTF.Exp, bias=0.0, scale=scale
                    )
                    if w % 2 == 1:
                        pending.append(emit_reduce_group(w // 2))
            # drain pending mm2 groups with ~2 steps of lag so the PE never
            # parks on an r3 wait ahead of ready mm1 work
            while pending and 3 * (mm2_emitted[0] // 3) + 4 <= s:
                emit_mm2_group(pending.pop(0))
            for fn in sched[s]:
                fn()
        # tail: window 42 holds the final chunk (1024 cols)
        nc.scalar.activation(
            K2r[:, 42 % 4, 0:1024], gw[:, 0:1024], ACTF.Exp, bias=0.0, scale=scale
        )
        base = (42 * WIN) % 6144
        kv = K2f[:, base : base + 1024]
        r1t = work.tile([128, 512], FP16, tag="r1t")
        nc.vector.tensor_add(r1t, kv[:, 0:512], kv[:, 512:1024])
        r2t = work.tile([128, 256], FP16, tag="r2t")
        nc.vector.tensor_add(r2t, r1t[:, 0:256], r1t[:, 256:512])
        r3t = work.tile([128, 128], FP16, tag="r3t")
        nc.vector.tensor_add(r3t, r2t[:, 0:128], r2t[:, 128:256])
        for entry in pending:
            emit_mm2_group(entry)
        emit_mm2_group((r3t.rearrange("p (o f) -> p o f", o=1), [TOT_STEPS - 1]))

        # ---- epilogue: T = reduce(S), probs = T[:, :10] / T[:, 10] --------
        Tred = const.tile([DY + 1, BPC], FP32)
        nc.vector.tensor_reduce(
            out=Tred.rearrange("p (t o) -> p t o", o=1),
            in_=S.rearrange("p (t f) -> p t f", f=4),
            axis=AX.X,
            op=ALU.add,
        )
        trT = psum.tile([BPC, DY + 1], FP32, tag="trk", bufs=1)
        nc.tensor.transpose(trT, Tred, ident32)
        Tt = const.tile([BPC, DY + 1], FP32)
        nc.vector.tensor_copy(Tt, trT)
        recd = const.tile([BPC, 1], FP32)
        nc.vector.reciprocal(recd, Tt[:, DY : DY + 1])
        outsb = const.tile([BPC, DY], FP32)
        nc.vector.tensor_scalar(
            out=outsb, in0=Tt[:, 0:DY], scalar1=recd, scalar2=None, op0=ALU.mult
        )
        nc.sync.dma_start(out=out_d, in_=outsb)


def build_program(scale):
    nc = bacc.Bacc(
        "TRN2",
        target_bir_lowering=False,
        debug=False,
        enable_asserts=False,
        num_devices=NCORES,
    )
    inp = nc.dram_tensor("inputs", [BPC, N, DX], FP32, kind="ExternalInput").ap()
    cx = nc.dram_tensor("c_x", [M, DX], FP32, kind="ExternalInput").ap()
    cy = nc.dram_tensor("c_y", [M, DY], FP32, kind="ExternalInput").ap()
    cw = nc.dram_tensor("comp_w", [M], FP32, kind="ExternalInput").ap()
    out = nc.dram_tensor("out", [BPC, DY], FP32, kind="ExternalOutput").ap()
    with tile.TileContext(nc) as tc:
        _body(tc, inp, cx, cy, cw, out, scale)
    nc.compile()
    return nc


_PROGRAM_CACHE: dict = {}


def _get_program(scale):
    nc = _PROGRAM_CACHE.get(scale)
    if nc is None:
        nc = build_program(scale)
        _PROGRAM_CACHE[scale] = nc
    return nc


def make_in_maps(inputs, c_x, c_y, comp_w):
    shards = np.ascontiguousarray(inputs.reshape(NCORES, BPC, N, DX))
    return [
        {
            "inputs": shards[i],
            "c_x": np.ascontiguousarray(c_x),
            "c_y": np.ascontiguousarray(c_y),
            "comp_w": np.ascontiguousarray(comp_w),
        }
        for i in range(NCORES)
    ]


def scale_from_sigma(sigma) -> float:
    s = max(float(np.asarray(sigma, dtype=np.float64)), MIN_SIGMA)
    return float(2.0 / (s * s))


def kernel(inputs, sigma, c_x, c_y, comp_w, _run_kwargs=None):
    nc = _get_program(scale_from_sigma(sigma))
    in_maps = make_in_maps(inputs, c_x, c_y, comp_w)
    res = run_bass_kernel_spmd(
        nc, in_maps, core_ids=list(range(NCORES)), **(_run_kwargs or {})
    )
    out = np.concatenate([res.results[i]["out"] for i in range(NCORES)], axis=0)
    return out.astype(np.float32)


# revision 6
# speedup vs baseline: 1.2503x; 1.2503x over previous
"""Trainium2 Bass kernel for nn_BagKQMClassModel.

Computation (per batch item b):
    K[b,n,m]   = exp(-d2/(2 s^2)),  d2 = |A[b,n] - C[m]|^2
    out_w[b,m] = (1/N) sum_n comp_w[m] * K^2
    y_w        = out_w / sum_m out_w
    probs      = y_w @ (y_v^2),  y_v = c_y rows normalized

Key transformations:
  * K^2 = exp(-d2/s^2): one exp per (b,n,m) element.
  * d2 folded into one fp16 matmul with augmented contraction (34 rows):
        rows 0-31: data a_d * c_d;  row 32: CT 1, AT -a2/2;
        row 33: CT -b2/2, AT 1
    so exp arg = (2/s^2) * g with no ACT bias.
  * probs = T[:, :10] / T[:, 10], T = sum_{n,m} K2 * W with
    W[m, :10] = comp_w[m] * c_y[m]^2 / |c_y[m]|^2, W[m, 10] = comp_w[m].
  * m and (b,n) are PERMUTED vs the reference layout so every input DMA
    is contiguous (partition p holds a contiguous DRAM block):
    m = 16p + r;  bn: p = 4t + n//32, j = n%32.  All downstream sums are
    permutation-invariant since c_y/comp_w use the same m-permutation.
  * One PSUM tile S[11, 128] accumulates mm2 over all 64 (m-chunk,
    j-block) steps; the final per-batch output is one [11, 32, 4]
    free-dim reduce + an 11x32 transpose.
  * exp runs on 1536-wide ACT windows (3 mm1 pieces of 512) from a
    2-deep PSUM ring (3 banks each; + 1 bank S + 1 bank transpose
    scratch = 8 banks), amortizing the ~300-cycle ACT instr overhead.
  * The 8:1 bag reduction batches SIX chunks (4 windows = one
    contiguous 6144-col span of the 8-slot K2 ring) into one DVE op per
    halving level, amortizing the ~151-cycle DVE op overhead.
  * The PE reaches its 2.4 GHz p-state when kept continuously busy
    (measured: back-to-back 512-col fp16 matmuls pace at 216 ns after
    ~10 instructions, all 8 cores active).  The loop keeps the PE dense:
    mm1 pieces, per-chunk mm2, and the deferred CT/AT transposes for
    chunks 2-15 / j-blocks 2-3 all run inside the loop.

Sharding: batch 256 -> 32 items per core across 8 cores; c_x/c_y/comp_w
replicated. No collectives (forward only).
"""

import numpy as np

import concourse.bacc as bacc
import concourse.mybir as mybir
import concourse.tile as tile
from concourse.bass_utils import run_bass_kernel_spmd
from concourse.masks import make_identity

NCORES = 8
BS, N, DX, DY, M = 256, 128, 32, 10, 2048
BPC = BS // NCORES      # 32 batch items per core
MB = M // 128           # 16 chunks of the component axis
KAUG = DX + 2           # 34 augmented contraction rows
NJ = 32                 # bn = 32 j-slices x 128 p
JB = 8                  # j slices per block
NBLK = NJ // JB         # 4 blocks
F_CHUNK = JB * 128      # 1024 bn columns per (m-chunk, j-block) step
WIN = 1536              # ACT window (3 x 512-col mm1 pieces)
NRING = 8               # K2 ring slots (6-chunk reduce groups never wrap)
GRP = 6                 # chunks per DVE reduce group (= 4 windows)
MIN_SIGMA = 1e-3
FP32 = mybir.dt.float32
FP16 = mybir.dt.float16
AX = mybir.AxisListType
ALU = mybir.AluOpType
ACTF = mybir.ActivationFunctionType


def _body(tc, inp, cx, cy, cw_d, out_d, scale):
    nc = tc.nc
    from contextlib import ExitStack

    with ExitStack() as ctx:
        const = ctx.enter_context(tc.tile_pool(name="const", bufs=1))
        work = ctx.enter_context(tc.tile_pool(name="work", bufs=2))
        psum = ctx.enter_context(tc.tile_pool(name="psum", bufs=1, space="PSUM"))

        # ---- contiguous input DMAs ----------------------------------------
        A_nat = const.tile([128, NJ, DX], FP32)     # p = 4t + n//32, j = n%32
        inp_r = inp.rearrange("t (a j) d -> (t a) j d", a=4)
        nc.sync.dma_start(out=A_nat[:, 0:16, :], in_=inp_r[:, 0:16, :])
        nc.scalar.dma_start(out=A_nat[:, 16:32, :], in_=inp_r[:, 16:32, :])
        cx_nat = const.tile([128, MB, DX], FP32)    # m = 16p + r
        nc.sync.dma_start(out=cx_nat, in_=cx.rearrange("(p r) d -> p r d", p=128))
        cy_nat = const.tile([128, MB, DY], FP32)
        nc.gpsimd.dma_start(out=cy_nat, in_=cy.rearrange("(p r) d -> p r d", p=128))
        cw_nat = const.tile([128, MB], FP32)
        nc.gpsimd.dma_start(out=cw_nat, in_=cw_d.rearrange("(p r) -> p r", p=128))

        ident16 = const.tile([128, 128], FP16)
        make_identity(nc, ident16)
        ident32 = const.tile([DY + 1, DY + 1], FP32)
        make_identity(nc, ident32)

        # preload the exp table set during the prologue DMA wait
        warm = const.tile([128, 1], FP32)
        nc.gpsimd.memset(warm, 0.0)
        warm2 = const.tile([128, 1], FP32)
        nc.scalar.activation(warm2, warm, ACTF.Exp, bias=0.0, scale=1.0)

        # packed fp16 transpose sources
        cx_pack = const.tile([128, MB, KAUG], FP16)  # [d x32, 1.0, -b2/2]
        A_pack = const.tile([128, NJ, KAUG], FP16)   # [d x32, -a2/2, 1.0]
        nc.gpsimd.memset(cx_pack[:, :, DX : DX + 1], 1.0)
        nc.gpsimd.memset(A_pack[:, :, DX + 1 : DX + 2], 1.0)

        CT16 = const.tile([KAUG, MB, 128], FP16)
        AT16 = const.tile([KAUG, NJ, 128], FP16)
        W16 = const.tile([128, MB, DY + 1], FP16)
        S = psum.tile([DY + 1, 128], FP32, tag="S")
        K2r = const.tile([128, NRING, WIN], FP16)
        K2f = K2r.rearrange("p w f -> p (w f)")

        one3 = lambda t: t.rearrange("p (s o) -> p s o", o=1)

        # ---- prep helpers --------------------------------------------------
        def quant_chain(nat, pack, lo, hi, colh):
            sq = work.tile([128, NJ, DX], FP32, tag="sq", bufs=2)
            sqv = sq[:, 0 : hi - lo, :]
            nc.vector.tensor_mul(sqv, nat[:, lo:hi, :], nat[:, lo:hi, :])
            mh = work.tile([128, NJ], FP32, tag="mh", bufs=2)
            mhv = mh[:, 0 : hi - lo]
            nc.vector.tensor_reduce(out=one3(mhv), in_=sqv, axis=AX.X, op=ALU.add)
            nc.vector.tensor_scalar_mul(mhv, mhv, -0.5)
            nc.vector.tensor_copy(pack[:, lo:hi, colh : colh + 1], one3(mhv))
            nc.vector.tensor_copy(pack[:, lo:hi, 0:DX], nat[:, lo:hi, :])

        def transpose_one(pack, dst, idx, use_scalar):
            trk = psum.tile([KAUG, 128], FP16, tag="trk", bufs=1)
            nc.tensor.transpose(trk, pack[:, idx, :], ident16)
            if use_scalar:
                nc.scalar.copy(dst[:, idx, :], trk)
            else:
                nc.vector.tensor_copy(dst[:, idx, :], trk)

        def w_chain():
            sqy = work.tile([128, MB, DY], FP32, tag="sqy")
            nc.vector.tensor_mul(sqy, cy_nat, cy_nat)
            ssum = work.tile([128, MB], FP32, tag="ssum")
            nc.vector.tensor_reduce(out=one3(ssum), in_=sqy, axis=AX.X, op=ALU.add)
            rec = work.tile([128, MB], FP32, tag="rec")
            nc.vector.reciprocal(rec, ssum)
            facr = work.tile([128, MB], FP32, tag="facr")
            nc.vector.tensor_mul(facr, rec, cw_nat)
            facr_b = one3(facr).broadcast_to([128, MB, DY])
            wtmp = work.tile([128, MB, DY], FP32, tag="wtmp")
            nc.vector.tensor_mul(wtmp, sqy, facr_b)
            nc.vector.tensor_copy(W16[:, :, 0:DY], wtmp)
            nc.vector.tensor_copy(W16[:, :, DY : DY + 1], one3(cw_nat))

        # ---- prologue: cx chain, CT 0-1, W, AT blocks 0-1 -----------------
        quant_chain(cx_nat, cx_pack, 0, MB, DX + 1)
        for c in range(2):
            transpose_one(cx_pack, CT16, c, True)
        quant_chain(A_nat, A_pack, 0, JB, DX)
        for j in range(JB):
            transpose_one(A_pack, AT16, j, True)
        w_chain()
        quant_chain(A_nat, A_pack, JB, 2 * JB, DX)
        for j in range(JB, 2 * JB):
            transpose_one(A_pack, AT16, j, True)

        # deferred prep: one quantum per step, deadline-ordered.
        quanta = []
        for c in range(2, MB):           # used at step c (jb=0)
            quanta.append(lambda c=c: transpose_one(cx_pack, CT16, c, False))
        for blk in (2, 3):               # block b used from step 16b
            jlo = blk * JB
            quanta.append(lambda jlo=jlo: quant_chain(A_nat, A_pack, jlo, jlo + JB, DX))
            for j in range(jlo, jlo + JB):
                quanta.append(lambda j=j: transpose_one(A_pack, AT16, j, False))
        sched = {s: [quanta[s]] if s < len(quanta) else [] for s in range(64)}

        AT16f = AT16.rearrange("p j c -> p (j c)")
        TOT = NBLK * MB

        # ---- main loop ----------------------------------------------------
        def emit_reduce(grp_base, nch):
            kv = K2f[:, grp_base : grp_base + nch * F_CHUNK].rearrange(
                "p (t f) -> p t f", t=nch
            )
            r1 = work.tile([128, GRP, 512], FP16, tag="r1")
            r1v = r1[:, 0:nch, :]
            nc.vector.tensor_add(r1v, kv[:, :, 0:512], kv[:, :, 512:1024])
            r2 = work.tile([128, GRP, 256], FP16, tag="r2")
            r2v = r2[:, 0:nch, :]
            nc.vector.tensor_add(r2v, r1v[:, :, 0:256], r1v[:, :, 256:512])
            r3 = work.tile([128, GRP, 128], FP16, tag="r3", bufs=3)
            r3v = r3[:, 0:nch, :]
            nc.vector.tensor_add(r3v, r2v[:, :, 0:128], r2v[:, :, 128:256])
            return r3

        def emit_mm2(entry):
            r3t, s0, nch = entry
            for i in range(nch):
                s_ = s0 + i
                nc.tensor.matmul(
                    S,
                    W16[:, s_ % MB, :],
                    r3t[:, i, :],
                    start=(s_ == 0),
                    stop=(s_ == TOT - 1),
                )
            return s0 + nch

        pieces = 0
        gw = None
        pending = []
        mm2_done = 0
        for s in range(TOT):
            jb, c = divmod(s, MB)
            for q in range(2):
                if pieces % 3 == 0:
                    gw = psum.tile([128, WIN], FP32, tag="g", bufs=2, name="gw")
                off = (pieces * 512) % WIN
                nc.tensor.matmul(
                    gw[:, off : off + 512],
                    CT16[:, c, :],
                    AT16f[:, jb * F_CHUNK + q * 512 : jb * F_CHUNK + (q + 1) * 512],
                    start=True,
                    stop=True,
                )
                pieces += 1
                if pieces % 3 == 0:
                    w = pieces // 3 - 1
                    nc.scalar.activation(
                        K2r[:, w % NRING, :], gw, ACTF.Exp, bias=0.0, scale=scale
                    )
                    if w % 4 == 3:
                        grp = w // 4
                        r3t = emit_reduce(6144 * (grp % 2), GRP)
                        pending.append((r3t, GRP * grp, GRP))
            while pending and pending[0][1] + pending[0][2] + 2 <= s:
                mm2_done = emit_mm2(pending.pop(0))
            for fn in sched[s]:
                fn()
        # tail: pieces 120..127 -> windows 40, 41 (full) + 42 (1024 cols)
        nc.scalar.activation(
            K2r[:, 42 % NRING, 0:1024], gw[:, 0:1024], ACTF.Exp, bias=0.0, scale=scale
        )
        r3t = emit_reduce((40 * WIN) % (NRING * WIN), 4)
        pending.append((r3t, 60, 4))
        for entry in pending:
            mm2_done = emit_mm2(entry)
        assert mm2_done == TOT

        # ---- epilogue: T = reduce(S), probs = T[:, :10] / T[:, 10] --------
        Tred = const.tile([DY + 1, BPC], FP32)
        nc.vector.tensor_reduce(
            out=Tred.rearrange("p (t o) -> p t o", o=1),
            in_=S.rearrange("p (t f) -> p t f", f=4),
            axis=AX.X,
            op=ALU.add,
        )
        trT = psum.tile([BPC, DY + 1], FP32, tag="trk", bufs=1)
        nc.tensor.transpose(trT, Tred, ident32)
        Tt = const.tile([BPC, DY + 1], FP32)
        nc.vector.tensor_copy(Tt, trT)
        recd = const.tile([BPC, 1], FP32)
        nc.vector.reciprocal(recd, Tt[:, DY : DY + 1])
        outsb = const.tile([BPC, DY], FP32)
        nc.vector.tensor_scalar(
            out=outsb, in0=Tt[:, 0:DY], scalar1=recd, scalar2=None, op0=ALU.mult
        )
        nc.sync.dma_start(out=out_d, in_=outsb)


def build_program(scale):
    nc = bacc.Bacc(
        "TRN2",
        target_bir_lowering=False,
        debug=False,
        enable_asserts=False,
        num_devices=NCORES,
    )
    inp = nc.dram_tensor("inputs", [BPC, N, DX], FP32, kind="ExternalInput").ap()
    cx = nc.dram_tensor("c_x", [M, DX], FP32, kind="ExternalInput").ap()
    cy = nc.dram_tensor("c_y", [M, DY], FP32, kind="ExternalInput").ap()
    cw = nc.dram_tensor("comp_w", [M], FP32, kind="ExternalInput").ap()
    out = nc.dram_tensor("out", [BPC, DY], FP32, kind="ExternalOutput").ap()
    with tile.TileContext(nc) as tc:
        _body(tc, inp, cx, cy, cw, out, scale)
    nc.compile()
    return nc


_PROGRAM_CACHE: dict = {}


def _get_program(scale):
    nc = _PROGRAM_CACHE.get(scale)
    if nc is None:
        nc = build_program(scale)
        _PROGRAM_CACHE[scale] = nc
    return nc


def make_in_maps(inputs, c_x, c_y, comp_w):
    shards = np.ascontiguousarray(inputs.reshape(NCORES, BPC, N, DX))
    return [
        {
            "inputs": shards[i],
            "c_x": np.ascontiguousarray(c_x),
            "c_y": np.ascontiguousarray(c_y),
            "comp_w": np.ascontiguousarray(comp_w),
        }
        for i in range(NCORES)
    ]


def scale_from_sigma(sigma) -> float:
    s = max(float(np.asarray(sigma, dtype=np.float64)), MIN_SIGMA)
    return float(2.0 / (s * s))


def kernel(inputs, sigma, c_x, c_y, comp_w, _run_kwargs=None):
    nc = _get_program(scale_from_sigma(sigma))
    in_maps = make_in_maps(inputs, c_x, c_y, comp_w)
    res = run_bass_kernel_spmd(
        nc, in_maps, core_ids=list(range(NCORES)), **(_run_kwargs or {})
    )
    out = np.concatenate([res.results[i]["out"] for i in range(NCORES)], axis=0)
    return out.astype(np.float32)


# revision 12
# speedup vs baseline: 1.3329x; 1.0660x over previous
"""Trainium2 Bass kernel for nn_BagKQMClassModel.

Computation (per batch item b):
    K[b,n,m]   = exp(-d2/(2 s^2)),  d2 = |A[b,n] - C[m]|^2
    out_w[b,m] = (1/N) sum_n comp_w[m] * K^2
    y_w        = out_w / sum_m out_w
    probs      = y_w @ (y_v^2),  y_v = c_y rows normalized

Key transformations:
  * K^2 = exp(-d2/s^2): one exp per (b,n,m) element.
  * d2 folded into one fp16 matmul with augmented contraction (34 rows):
        rows 0-31: data a_d * c_d;  row 32: CT 1, AT -a2/2;
        row 33: CT -b2/2, AT 1
    so exp arg = (2/s^2) * g with no ACT bias.
  * probs = T[:, :10] / T[:, 10], T = sum_{n,m} K2 * W with
    W[m, :10] = comp_w[m] * c_y[m]^2 / |c_y[m]|^2, W[m, 10] = comp_w[m].
  * m and (b,n) are PERMUTED vs the reference layout so every input DMA
    is contiguous (partition p holds a contiguous DRAM block):
    m = 16p + r;  bn: p = 4t + n//32, j = n%32.  All downstream sums are
    permutation-invariant since c_y/comp_w use the same m-permutation.
  * One PSUM tile S[11, 128] accumulates mm2 over all 64 (m-chunk,
    j-block) steps; the final per-batch output is one [11, 32, 4]
    free-dim reduce + an 11x32 transpose.
  * exp runs on 1536-wide ACT windows (3 mm1 pieces of 512) from a
    2-deep PSUM ring (3 banks each; + 1 bank S + 1 bank transpose
    scratch = 8 banks), amortizing the ~300-cycle ACT instr overhead.
  * The 8:1 bag reduction batches SIX chunks (4 windows = one
    contiguous 6144-col span of the 8-slot K2 ring) into one DVE op per
    halving level, amortizing the ~151-cycle DVE op overhead.
  * The PE reaches its 2.4 GHz p-state when kept continuously busy
    (measured: back-to-back 512-col fp16 matmuls pace at 216 ns after
    ~10 instructions, all 8 cores active).  The loop keeps the PE dense:
    mm1 pieces, per-chunk mm2, and the deferred CT/AT transposes for
    chunks 2-15 / j-blocks 2-3 all run inside the loop.

Sharding: batch 256 -> 32 items per core across 8 cores; c_x/c_y/comp_w
replicated. No collectives (forward only).
"""

import numpy as np

import concourse.bacc as bacc
import concourse.mybir as mybir
import concourse.tile as tile
from concourse.bass_utils import run_bass_kernel_spmd
from concourse.masks import make_identity

NCORES = 8
BS, N, DX, DY, M = 256, 128, 32, 10, 2048
BPC = BS // NCORES      # 32 batch items per core
MB = M // 128           # 16 chunks of the component axis
KAUG = DX + 2           # 34 augmented contraction rows
NJ = 32                 # bn = 32 j-slices x 128 p
JB = 8                  # j slices per block
NBLK = NJ // JB         # 4 blocks
F_CHUNK = JB * 128      # 1024 bn columns per (m-chunk, j-block) step
WIN = 1024              # ACT window = one chunk (2 x 512-col mm1 pieces)
NRING = 12              # K2 ring slots (6-chunk reduce groups never wrap)
GRP = 6                 # chunks per DVE reduce group
MIN_SIGMA = 1e-3
FP32 = mybir.dt.float32
FP16 = mybir.dt.float16
AX = mybir.AxisListType
ALU = mybir.AluOpType
ACTF = mybir.ActivationFunctionType


def _body(tc, inp, cx, cy, cw_d, out_d, scale):
    nc = tc.nc
    from contextlib import ExitStack

    with ExitStack() as ctx:
        const = ctx.enter_context(tc.tile_pool(name="const", bufs=1))
        work = ctx.enter_context(tc.tile_pool(name="work", bufs=2))
        psum = ctx.enter_context(tc.tile_pool(name="psum", bufs=1, space="PSUM"))

        # ---- contiguous input DMAs ----------------------------------------
        A_nat = const.tile([128, NJ, DX], FP32)     # p = 4t + n//32, j = n%32
        inp_r = inp.rearrange("t (a j) d -> (t a) j d", a=4)
        nc.sync.dma_start(out=A_nat[:, 0:16, :], in_=inp_r[:, 0:16, :])
        nc.scalar.dma_start(out=A_nat[:, 16:32, :], in_=inp_r[:, 16:32, :])
        cx_nat = const.tile([128, MB, DX], FP32)    # m = 16p + r
        nc.sync.dma_start(out=cx_nat, in_=cx.rearrange("(p r) d -> p r d", p=128))
        cy_nat = const.tile([128, MB, DY], FP32)
        nc.gpsimd.dma_start(out=cy_nat, in_=cy.rearrange("(p r) d -> p r d", p=128))
        cw_nat = const.tile([128, MB], FP32)
        nc.gpsimd.dma_start(out=cw_nat, in_=cw_d.rearrange("(p r) -> p r", p=128))

        ident16 = const.tile([128, 128], FP16)
        make_identity(nc, ident16)
        ident32 = const.tile([DY + 1, DY + 1], FP32)
        make_identity(nc, ident32)

        # preload the exp table set during the prologue DMA wait
        warm = const.tile([128, 1], FP32)
        nc.gpsimd.memset(warm, 0.0)
        warm2 = const.tile([128, 1], FP32)
        nc.scalar.activation(warm2, warm, ACTF.Exp, bias=0.0, scale=1.0)

        # packed fp16 transpose sources
        cx_pack = const.tile([128, MB, KAUG], FP16)  # [d x32, 1.0, -b2/2]
        A_pack = const.tile([128, NJ, KAUG], FP16)   # [d x32, -a2/2, 1.0]
        nc.gpsimd.memset(cx_pack[:, :, DX : DX + 1], 1.0)
        nc.gpsimd.memset(A_pack[:, :, DX + 1 : DX + 2], 1.0)

        CT16 = const.tile([KAUG, MB, 128], FP16)
        AT16 = const.tile([KAUG, NJ, 128], FP16)
        W16 = const.tile([128, MB, DY + 1], FP16)
        S = psum.tile([DY + 1, 128], FP32, tag="S")
        K2r = const.tile([128, NRING, WIN], FP16)
        K2f = K2r.rearrange("p w f -> p (w f)")

        one3 = lambda t: t.rearrange("p (s o) -> p s o", o=1)

        # ---- prep helpers --------------------------------------------------
        def quant_chain(nat, pack, lo, hi, colh):
            sq = work.tile([128, NJ, DX], FP32, tag="sq", bufs=2)
            sqv = sq[:, 0 : hi - lo, :]
            nc.vector.tensor_mul(sqv, nat[:, lo:hi, :], nat[:, lo:hi, :])
            mh = work.tile([128, NJ], FP32, tag="mh", bufs=2)
            mhv = mh[:, 0 : hi - lo]
            nc.vector.tensor_reduce(out=one3(mhv), in_=sqv, axis=AX.X, op=ALU.add)
            nc.vector.tensor_scalar_mul(mhv, mhv, -0.5)
            nc.vector.tensor_copy(pack[:, lo:hi, colh : colh + 1], one3(mhv))
            nc.vector.tensor_copy(pack[:, lo:hi, 0:DX], nat[:, lo:hi, :])

        def transpose_one(pack, dst, idx, use_scalar):
            trk = psum.tile([KAUG, 128], FP16, tag="trk", bufs=3)
            nc.tensor.transpose(trk, pack[:, idx, :], ident16)
            if use_scalar:
                nc.scalar.copy(dst[:, idx, :], trk)
            else:
                nc.vector.tensor_copy(dst[:, idx, :], trk)

        def w_chain():
            sqy = work.tile([128, MB, DY], FP32, tag="sqy")
            nc.vector.tensor_mul(sqy, cy_nat, cy_nat)
            ssum = work.tile([128, MB], FP32, tag="ssum")
            nc.vector.tensor_reduce(out=one3(ssum), in_=sqy, axis=AX.X, op=ALU.add)
            rec = work.tile([128, MB], FP32, tag="rec")
            nc.vector.reciprocal(rec, ssum)
            facr = work.tile([128, MB], FP32, tag="facr")
            nc.vector.tensor_mul(facr, rec, cw_nat)
            facr_b = one3(facr).broadcast_to([128, MB, DY])
            wtmp = work.tile([128, MB, DY], FP32, tag="wtmp")
            nc.vector.tensor_mul(wtmp, sqy, facr_b)
            nc.vector.tensor_copy(W16[:, :, 0:DY], wtmp)
            nc.vector.tensor_copy(W16[:, :, DY : DY + 1], one3(cw_nat))

        # ---- prologue: cx chain, CT 0-1, AT block 0 -----------------------
        quant_chain(cx_nat, cx_pack, 0, MB, DX + 1)
        for c in range(2):
            transpose_one(cx_pack, CT16, c, True)
        quant_chain(A_nat, A_pack, 0, JB, DX)
        for j in range(JB):
            transpose_one(A_pack, AT16, j, True)

        # deferred prep, deadline-ordered: CT chunk c used at step c; AT
        # block b used from step 16b; W used by the first mm2 drain (~s8).
        def ct_q(c):
            return lambda: transpose_one(cx_pack, CT16, c, False)

        def at_q(j):
            return lambda: transpose_one(A_pack, AT16, j, False)

        def chain_q(jlo):
            return lambda: quant_chain(A_nat, A_pack, jlo, jlo + JB, DX)

        quanta = [ct_q(2), ct_q(3), ct_q(4), w_chain, ct_q(5), chain_q(JB)]
        for c in range(6, 14):
            quanta += [ct_q(c), at_q(c + 2)]     # AT block 1: j = 8..15
        quanta += [ct_q(14), ct_q(15)]
        for blk in (2, 3):
            jlo = blk * JB
            quanta.append(chain_q(jlo))
            quanta += [at_q(j) for j in range(jlo, jlo + JB)]
        # 2 quanta/step for the first 13 steps, then 1/step (done by s28).
        sched = {s: [] for s in range(64)}
        qi = 0
        for s in range(64):
            take = min(2 if s < 13 else 1, len(quanta) - qi)
            sched[s] = quanta[qi : qi + take]
            qi += take
        assert qi == len(quanta), (qi, len(quanta))

        AT16f = AT16.rearrange("p j c -> p (j c)")
        TOT = NBLK * MB

        # ---- main loop ----------------------------------------------------
        def emit_reduce(grp_base, nch):
            kv = K2f[:, grp_base : grp_base + nch * F_CHUNK].rearrange(
                "p (t f) -> p t f", t=nch
            )
            r1 = work.tile([128, GRP, 512], FP16, tag="r1")
            r1v = r1[:, 0:nch, :]
            nc.vector.tensor_add(r1v, kv[:, :, 0:512], kv[:, :, 512:1024])
            r2 = work.tile([128, GRP, 256], FP16, tag="r2")
            r2v = r2[:, 0:nch, :]
            nc.vector.tensor_add(r2v, r1v[:, :, 0:256], r1v[:, :, 256:512])
            r3 = work.tile([128, GRP, 128], FP16, tag="r3", bufs=3)
            r3v = r3[:, 0:nch, :]
            nc.vector.tensor_add(r3v, r2v[:, :, 0:128], r2v[:, :, 128:256])
            return r3

        def emit_mm2(entry):
            r3t, s0, nch = entry
            for i in range(nch):
                s_ = s0 + i
                nc.tensor.matmul(
                    S,
                    W16[:, s_ % MB, :],
                    r3t[:, i, :],
                    start=(s_ == 0),
                    stop=(s_ == TOT - 1),
                )
            return s0 + nch

        pending = []
        mm2_done = 0
        for s in range(TOT):
            jb, c = divmod(s, MB)
            gw = psum.tile([128, WIN], FP32, tag="g", bufs=2, name="gw")
            for q in range(2):
                nc.tensor.matmul(
                    gw[:, q * 512 : (q + 1) * 512],
                    CT16[:, c, :],
                    AT16f[:, jb * F_CHUNK + q * 512 : jb * F_CHUNK + (q + 1) * 512],
                    start=True,
                    stop=True,
                )
            nc.scalar.activation(
                K2r[:, s % NRING, :], gw, ACTF.Exp, bias=0.0, scale=scale
            )
            if s % GRP == GRP - 1 and s < GRP * (TOT // GRP):
                grp = s // GRP
                r3t = emit_reduce(WIN * ((GRP * grp) % NRING), GRP)
                pending.append((r3t, GRP * grp, GRP))
            while pending and pending[0][1] + pending[0][2] + 2 <= s:
                mm2_done = emit_mm2(pending.pop(0))
            for fn in sched[s]:
                fn()
        # tail: chunks 60..63 (ring slots 0..3)
        r3t = emit_reduce(WIN * (60 % NRING), 4)
        pending.append((r3t, 60, 4))
        for entry in pending:
            mm2_done = emit_mm2(entry)
        assert mm2_done == TOT

        # ---- epilogue: T = reduce(S), probs = T[:, :10] / T[:, 10] --------
        Tred = const.tile([DY + 1, BPC], FP32)
        nc.vector.tensor_reduce(
            out=Tred.rearrange("p (t o) -> p t o", o=1),
            in_=S.rearrange("p (t f) -> p t f", f=4),
            axis=AX.X,
            op=ALU.add,
        )
        trT = psum.tile([BPC, DY + 1], FP32, tag="trk", bufs=3)
        nc.tensor.transpose(trT, Tred, ident32)
        Tt = const.tile([BPC, DY + 1], FP32)
        nc.vector.tensor_copy(Tt, trT)
        recd = const.tile([BPC, 1], FP32)
        nc.vector.reciprocal(recd, Tt[:, DY : DY + 1])
        outsb = const.tile([BPC, DY], FP32)
        nc.vector.tensor_scalar(
            out=outsb, in0=Tt[:, 0:DY], scalar1=recd, scalar2=None, op0=ALU.mult
        )
        nc.sync.dma_start(out=out_d, in_=outsb)


def build_program(scale):
    nc = bacc.Bacc(
        "TRN2",
        target_bir_lowering=False,
        debug=False,
        enable_asserts=False,
        num_devices=NCORES,
    )
    inp = nc.dram_tensor("inputs", [BPC, N, DX], FP32, kind="ExternalInput").ap()
    cx = nc.dram_tensor("c_x", [M, DX], FP32, kind="ExternalInput").ap()
    cy = nc.dram_tensor("c_y", [M, DY], FP32, kind="ExternalInput").ap()
    cw = nc.dram_tensor("comp_w", [M], FP32, kind="ExternalInput").ap()
    out = nc.dram_tensor("out", [BPC, DY], FP32, kind="ExternalOutput").ap()
    with tile.TileContext(nc) as tc:
        _body(tc, inp, cx, cy, cw, out, scale)
    nc.compile()
    return nc


_PROGRAM_CACHE: dict = {}


def _get_program(scale):
    nc = _PROGRAM_CACHE.get(scale)
    if nc is None:
        nc = build_program(scale)
        _PROGRAM_CACHE[scale] = nc
    return nc


def make_in_maps(inputs, c_x, c_y, comp_w):
    shards = np.ascontiguousarray(inputs.reshape(NCORES, BPC, N, DX))
    return [
        {
            "inputs": shards[i],
            "c_x": np.ascontiguousarray(c_x),
            "c_y": np.ascontiguousarray(c_y),
            "comp_w": np.ascontiguousarray(comp_w),
        }
        for i in range(NCORES)
    ]


def scale_from_sigma(sigma) -> float:
    s = max(float(np.asarray(sigma, dtype=np.float64)), MIN_SIGMA)
    return float(2.0 / (s * s))


def kernel(inputs, sigma, c_x, c_y, comp_w, _run_kwargs=None):
    nc = _get_program(scale_from_sigma(sigma))
    in_maps = make_in_maps(inputs, c_x, c_y, comp_w)
    res = run_bass_kernel_spmd(
        nc, in_maps, core_ids=list(range(NCORES)), **(_run_kwargs or {})
    )
    out = np.concatenate([res.results[i]["out"] for i in range(NCORES)], axis=0)
    return out.astype(np.float32)


# revision 13
# speedup vs baseline: 1.3379x; 1.0038x over previous
"""Trainium2 Bass kernel for nn_BagKQMClassModel.

Computation (per batch item b):
    K[b,n,m]   = exp(-d2/(2 s^2)),  d2 = |A[b,n] - C[m]|^2
    out_w[b,m] = (1/N) sum_n comp_w[m] * K^2
    y_w        = out_w / sum_m out_w
    probs      = y_w @ (y_v^2),  y_v = c_y rows normalized

Key transformations used here:
  * K^2 = exp(-d2/s^2), so only one exp per (b,n,m) element is needed.
  * d2 = a2[bn] + b2[m] - 2 g[m,bn] with g = C @ A^T.  Both -a2/2 and -b2/2
    are folded into the matmul by augmenting the contraction dim (K=34):
        row 32: C^T row = 1,       A^T row = -a2/2
        row 33: C^T row = -b2/2,   A^T row = 1
    so one matmul emits g_full with exp argument = (2/s^2) * g_full and the
    activation needs no bias at all.
  * probs = T[:, :10] / T[:, 10] where T[b,:] = sum_n sum_m K2[m,bn]*W[m,:],
    W[m, :10] = comp_w[m] * c_y[m]^2 / |c_y[m]|^2,  W[m, 10] = comp_w[m].
    The 1/N bag weight and normalization cancel.
  * Layout (m on partitions, b*n on free dim): matmul2 contracts m on the PE
    with W as the stationary operand; the final n-reduction happens on the
    tiny (11, bn) result via a segmented DVE reduce.

Sharding: batch 256 -> 32 items per core across 8 cores; c_x/c_y/comp_w
replicated. No collectives (forward only).
"""

import numpy as np

import concourse.bacc as bacc
import concourse.mybir as mybir
import concourse.tile as tile
from concourse.bass import ts
from concourse.bass_utils import run_bass_kernel_spmd
from concourse.masks import make_identity

NCORES = 8
BS, N, DX, DY, M = 256, 128, 32, 10, 2048
BPC = BS // NCORES      # 32 batch items per core
MB = M // 128           # 16 chunks of the component axis
KAUG = DX + 2           # 34: contraction dim with the two folded rows
NBLK = 4                # bn blocks per core
BLKI = BPC // NBLK      # 8 items per block
F_BLK = BLKI * N        # 1024 free elements per (block, m-chunk) tile
MIN_SIGMA = 1e-3
FP32 = mybir.dt.float32
BF16 = mybir.dt.bfloat16
FP16 = mybir.dt.float16
AX = mybir.AxisListType
ALU = mybir.AluOpType
ACTF = mybir.ActivationFunctionType


def _body(tc, inp, cx, cy, cw_d, out_d, scale):
    nc = tc.nc
    from contextlib import ExitStack

    with ExitStack() as ctx:
        const = ctx.enter_context(tc.tile_pool(name="const", bufs=1))
        work = ctx.enter_context(tc.tile_pool(name="work", bufs=2))
        k2p = ctx.enter_context(tc.tile_pool(name="k2p", bufs=4))
        psum = ctx.enter_context(tc.tile_pool(name="psum", bufs=2, space="PSUM"))

        identity = const.tile([128, 128], FP32)
        make_identity(nc, identity)
        identity_bf = const.tile([128, 128], FP16)
        make_identity(nc, identity_bf)

        # ---- input loads ---------------------------------------------------
        # c_x / inputs land in augmented (128, 34) layouts: cols 0:32 = data,
        # plus the 1.0 and -b2/2 (resp. -a2/2) columns, so transposing whole
        # tiles yields the augmented C^T/A^T rows without partition-offset
        # writes.  The gather DMAs are 128B-granule (slow), so they are
        # chunked to overlap with the per-chunk prep chains.
        cx_aug = const.tile([128, MB, KAUG], FP32)
        cx_r = cx.rearrange("(t p) d -> p t d", p=128)
        for c in range(2):
            nc.scalar.dma_start(
                out=cx_aug[:, ts(c, MB // 2), 0:DX], in_=cx_r[:, ts(c, MB // 2), :]
            )
        A_aug = const.tile([128, BPC, KAUG], FP32)
        inp_r = inp.rearrange("t p d -> p t d")
        for c in range(4):
            eng = nc.sync if c < 2 else nc.scalar
            eng.dma_start(
                out=A_aug[:, ts(c, BPC // 4), 0:DX], in_=inp_r[:, ts(c, BPC // 4), :]
            )
        nc.vector.memset(A_aug[:, :, DX + 1 : DX + 2], 1.0)
        nc.vector.memset(cx_aug[:, :, DX : DX + 1], 1.0)
        cy_all = const.tile([128, MB, DY], FP32)
        nc.gpsimd.dma_start(out=cy_all, in_=cy.rearrange("(t p) d -> p t d", p=128))
        cw_sb = const.tile([128, MB], FP32)
        nc.gpsimd.dma_start(out=cw_sb, in_=cw_d.rearrange("(t p) -> p t", p=128))

        # bf16 for both matmul operands: fp32 matmuls stream at 1/4 PE rate.
        CT = const.tile([KAUG, M], FP16)      # augmented C^T (stationary mm1)
        AT = const.tile([KAUG, BPC, N], FP16)  # augmented A^T (moving mm1)
        W_all = const.tile([128, MB, DY + 1], FP32)
        W_bf = const.tile([128, MB, DY + 1], BF16)
        T_sb = const.tile([DY + 1, BPC], FP32)


        # ---- chunked prep: square/reduce -> bf16 cast -> transposes -------
        # (fp16 transposes stream 4x faster through the PE than fp32 ones;
        # all psum->sbuf copies go to the DVE: the ACT engine is the kernel's
        # floor, so any copy there delays the exp stream 1:1)
        cx_bf = const.tile([128, MB, KAUG], FP16)
        A_bf = const.tile([128, BPC, KAUG], FP16)

        def prep_chunk(aug, bft, sq_tag, lo, hi, col):
            sqc = work.tile([128, hi - lo, DX], FP32, tag=sq_tag)
            nc.vector.tensor_mul(sqc, aug[:, lo:hi, 0:DX], aug[:, lo:hi, 0:DX])
            nc.vector.tensor_reduce(
                out=aug[:, lo:hi, col : col + 1], in_=sqc, axis=AX.X, op=ALU.add
            )
            nc.vector.tensor_scalar_mul(
                aug[:, lo:hi, col : col + 1], aug[:, lo:hi, col : col + 1], -0.5
            )
            nc.vector.tensor_copy(bft[:, lo:hi, :], aug[:, lo:hi, :])

        def transpose_to(bft, dst_is_ct, lo, hi):
            for k in range(lo, hi):
                trk = psum.tile([KAUG, 128], FP16, tag="jit", bufs=4)
                nc.tensor.transpose(trk, bft[:, k, :], identity_bf)
                dst = CT[:, ts(k, 128)] if dst_is_ct else AT[:, k, :]
                if k % 2 == 0:
                    nc.scalar.copy(dst, trk)
                else:
                    nc.vector.tensor_copy(dst, trk)

        for c in range(2):
            lo, hi = c * (MB // 2), (c + 1) * (MB // 2)
            prep_chunk(cx_aug, cx_bf, "sqx", lo, hi, DX + 1)
            transpose_to(cx_bf, True, lo, hi)
        # ---- W build (gates mm2 of the first loop step; emitted before the
        # A transposes so the DVE finishes it well ahead of the main loop) --
        sqy = work.tile([128, MB, DY], FP32, tag="sqy")
        nc.vector.tensor_mul(sqy, cy_all, cy_all)
        ssum = work.tile([128, MB], FP32, tag="ssum")
        nc.vector.tensor_reduce(out=ssum, in_=sqy, axis=AX.X, op=ALU.add)
        rec = work.tile([128, MB], FP32, tag="rec")
        nc.vector.reciprocal(rec, ssum)
        facr = work.tile([128, MB], FP32, tag="facr")
        nc.vector.tensor_mul(facr, rec, cw_sb)
        facr_b = facr.rearrange("p (t one) -> p t one", one=1).broadcast_to(
            [128, MB, DY]
        )
        nc.vector.tensor_mul(W_all[:, :, 0:DY], sqy, facr_b)
        nc.vector.tensor_copy(
            W_all[:, :, DY : DY + 1], cw_sb.rearrange("p (t one) -> p t one", one=1)
        )
        nc.vector.tensor_copy(W_bf, W_all)

        for c in range(4):
            lo, hi = c * (BPC // 4), (c + 1) * (BPC // 4)
            prep_chunk(A_aug, A_bf, "sqa", lo, hi, DX)
            transpose_to(A_bf, False, lo, hi)


        # ---- main pipeline ------------------------------------------------
        # The PE clock is power-limited to 1.2 GHz when all 8 cores run, so
        # the loop is balanced for that operating point: the DVE pre-reduces
        # K2 4:1 over n-pairs (valid because mm2 is linear in its free dim),
        # shrinking mm2's PE stream from 1024 to 256 columns per step.  Cold
        # PE (~1.2us) then matches the ACT exp (~1.1us) instead of pacing the
        # loop at ~1.8us.  mm2/reduce for step mb-1 are issued after mm1 for
        # step mb so the PE FIFO never blocks on the current step's ACT.
        F_R2 = F_BLK // 8

        def emit_reduce(k2):
            r1 = work.tile([128, F_BLK // 2], BF16, tag="r1")
            k2v = k2.rearrange("p (t two n) -> p t two n", two=2, n=N // 2)
            nc.vector.tensor_add(
                r1.rearrange("p (t n) -> p t n", n=N // 2),
                k2v[:, :, 0, :],
                k2v[:, :, 1, :],
            )
            r2 = work.tile([128, F_BLK // 4], BF16, tag="r2")
            r1v = r1.rearrange("p (t two n) -> p t two n", two=2, n=N // 4)
            nc.vector.tensor_add(
                r2.rearrange("p (t n) -> p t n", n=N // 4),
                r1v[:, :, 0, :],
                r1v[:, :, 1, :],
            )
            r3 = work.tile([128, F_R2], BF16, tag="r3")
            r2v = r2.rearrange("p (t two n) -> p t two n", two=2, n=N // 8)
            nc.vector.tensor_add(
                r3.rearrange("p (t n) -> p t n", n=N // 8),
                r2v[:, :, 0, :],
                r2v[:, :, 1, :],
            )
            return r3

        def emit_blk_reduce(S, blk):
            nc.vector.tensor_reduce(
                out=T_sb[:, blk * BLKI : (blk + 1) * BLKI],
                in_=S.rearrange("p (t n) -> p t n", n=N // 8),
                axis=AX.X,
                op=ALU.add,
            )

        # Flat loop: the mm2 software pipeline (one step behind mm1/exp)
        # carries across block boundaries so the engines never drain.
        r2_prev = None
        S_tiles = [None] * NBLK
        for j in range(NBLK * MB):
            blk, mb = divmod(j, MB)
            if mb == 0:
                S_tiles[blk] = psum.tile(
                    [DY + 1, F_R2], FP32, tag="jit", bufs=4, name=f"S{blk}"
                )
            g = psum.tile([128, F_BLK], FP32, tag="g", bufs=2)
            for q in range(F_BLK // 512):
                nc.tensor.matmul(
                    g[:, ts(q, 512)],
                    CT[:, ts(mb, 128)],
                    AT[:, blk * BLKI + q * 4 : blk * BLKI + (q + 1) * 4, :],
                    start=True,
                    stop=True,
                )
            if r2_prev is not None:
                pblk, pmb = divmod(j - 1, MB)
                nc.tensor.matmul(
                    S_tiles[pblk],
                    W_bf[:, pmb, :],
                    r2_prev,
                    start=(pmb == 0),
                    stop=(pmb == MB - 1),
                )
                if pmb == MB - 1:
                    emit_blk_reduce(S_tiles[pblk], pblk)
            K2 = k2p.tile([128, F_BLK], BF16, tag="k2")
            nc.scalar.activation(K2, g, ACTF.Exp, bias=0.0, scale=scale)
            r2_prev = emit_reduce(K2)
        last_blk, last_mb = NBLK - 1, MB - 1
        nc.tensor.matmul(
            S_tiles[last_blk], W_bf[:, last_mb, :], r2_prev, start=False, stop=True
        )
        emit_blk_reduce(S_tiles[last_blk], last_blk)

        # ---- epilogue: probs = T[:, :10] / T[:, 10] -----------------------
        trT = psum.tile([BPC, DY + 1], FP32, tag="jit", bufs=4)
        nc.tensor.transpose(trT, T_sb, identity[0 : DY + 1, 0 : DY + 1])
        Tt = const.tile([BPC, DY + 1], FP32)
        nc.vector.tensor_copy(Tt, trT)
        recd = const.tile([BPC, 1], FP32)
        nc.vector.reciprocal(recd, Tt[:, DY : DY + 1])
        outsb = const.tile([BPC, DY], FP32)
        nc.vector.tensor_scalar(
            out=outsb, in0=Tt[:, 0:DY], scalar1=recd, scalar2=None, op0=ALU.mult
        )
        nc.sync.dma_start(out=out_d, in_=outsb)


def build_program(scale):
    nc = bacc.Bacc(
        "TRN2",
        target_bir_lowering=False,
        debug=False,
        enable_asserts=False,
        num_devices=NCORES,
    )
    inp = nc.dram_tensor("inputs", [BPC, N, DX], FP32, kind="ExternalInput").ap()
    cx = nc.dram_tensor("c_x", [M, DX], FP32, kind="ExternalInput").ap()
    cy = nc.dram_tensor("c_y", [M, DY], FP32, kind="ExternalInput").ap()
    cw = nc.dram_tensor("comp_w", [M], FP32, kind="ExternalInput").ap()
    out = nc.dram_tensor("out", [BPC, DY], FP32, kind="ExternalOutput").ap()
    with tile.TileContext(nc) as tc:
        _body(tc, inp, cx, cy, cw, out, scale)
    nc.compile()
    return nc


_PROGRAM_CACHE: dict = {}


def _get_program(scale):
    nc = _PROGRAM_CACHE.get(scale)
    if nc is None:
        nc = build_program(scale)
        _PROGRAM_CACHE[scale] = nc
    return nc


def make_in_maps(inputs, c_x, c_y, comp_w):
    shards = np.ascontiguousarray(inputs.reshape(NCORES, BPC, N, DX))
    return [
        {
            "inputs": shards[i],
            "c_x": np.ascontiguousarray(c_x),
            "c_y": np.ascontiguousarray(c_y),
            "comp_w": np.ascontiguousarray(comp_w),
        }
        for i in range(NCORES)
    ]


def scale_from_sigma(sigma) -> float:
    s = max(float(np.asarray(sigma, dtype=np.float64)), MIN_SIGMA)
    return float(2.0 / (s * s))


def kernel(inputs, sigma, c_x, c_y, comp_w, _run_kwargs=None):
    nc = _get_program(scale_from_sigma(sigma))
    in_maps = make_in_maps(inputs, c_x, c_y, comp_w)
    res = run_bass_kernel_spmd(
        nc, in_maps, core_ids=list(range(NCORES)), **(_run_kwargs or {})
    )
    out = np.concatenate([res.results[i]["out"] for i in range(NCORES)], axis=0)
    return out.astype(np.float32)



# revision 16
# speedup vs baseline: 1.3875x; 1.0371x over previous
"""Trainium2 Bass kernel for nn_BagKQMClassModel.

Computation (per batch item b):
    K[b,n,m]   = exp(-d2/(2 s^2)),  d2 = |A[b,n] - C[m]|^2
    out_w[b,m] = (1/N) sum_n comp_w[m] * K^2
    y_w        = out_w / sum_m out_w
    probs      = y_w @ (y_v^2),  y_v = c_y rows normalized

Key transformations:
  * K^2 = exp(-d2/s^2): one exp per (b,n,m) element.
  * d2 folded into one fp16 matmul with augmented contraction (34 rows):
        rows 0-31: data a_d * c_d;  row 32: CT 1, AT -a2/2;
        row 33: CT -b2/2, AT 1
    so exp arg = (2/s^2) * g with no ACT bias.
  * probs = T[:, :10] / T[:, 10], T = sum_{n,m} K2 * W with
    W[m, :10] = comp_w[m] * c_y[m]^2 / |c_y[m]|^2, W[m, 10] = comp_w[m].
  * m and (b,n) are PERMUTED vs the reference layout so every input DMA
    is contiguous (partition p holds a contiguous DRAM block):
    m = 16p + r;  bn: p = 4t + n//32, j = n%32.  All downstream sums are
    permutation-invariant since c_y/comp_w use the same m-permutation.
  * One PSUM tile S[11, 128] accumulates mm2 over all 64 (m-chunk,
    j-block) steps; the final per-batch output is one [11, 32, 4]
    free-dim reduce + an 11x32 transpose.
  * mm2 runs as fp8e4 DoubleRow over CHUNK PAIRS (stationary W8
    [128, 2, 11], moving r3 [128, 2, 128]) - half the mm2 instruction
    count and half the stream cycles.  W8 is pre-scaled by 2048 so the
    tiny comp_w values stay inside fp8e4's dynamic range (min subnormal
    2^-9); the scale cancels in the final T[:, :10] / T[:, 10].
  * exp runs on 1024-wide ACT windows from a 2-deep PSUM ring (2 banks
    each; + 1 bank S + 3 x 1 bank transpose scratch = 8 banks).
  * The 8:1 bag reduction batches SIX chunks into one DVE op per
    halving level, amortizing the ~151-cycle DVE op overhead.
  * CT chunks 2-15, AT j-blocks 1-3, and the W build all run INSIDE the
    main loop in PE/DVE slack (deadline-scheduled quanta).

Sharding: batch 256 -> 32 items per core across 8 cores; c_x/c_y/comp_w
replicated. No collectives (forward only).
"""

import numpy as np

import concourse.bacc as bacc
import concourse.mybir as mybir
import concourse.tile as tile
from concourse.bass_utils import run_bass_kernel_spmd
from concourse.masks import make_identity

NCORES = 8
BS, N, DX, DY, M = 256, 128, 32, 10, 2048
BPC = BS // NCORES      # 32 batch items per core
MB = M // 128           # 16 chunks of the component axis
KAUG = DX + 2           # 34 augmented contraction rows
NJ = 32                 # bn = 32 j-slices x 128 p
JB = 8                  # j slices per block
NBLK = NJ // JB         # 4 blocks
F_CHUNK = JB * 128      # 1024 bn columns per (m-chunk, j-block) step
WIN = 1024              # ACT window = one chunk (2 x 512-col mm1 pieces)
NRING = 12              # K2 ring slots (6-chunk reduce groups never wrap)
GRP = 6                 # chunks per DVE reduce group
WSCL = 2048.0           # fp8 mm2 weight prescale (cancels in the division)
MIN_SIGMA = 1e-3
FP32 = mybir.dt.float32
FP16 = mybir.dt.float16
FP8 = mybir.dt.float8e4
AX = mybir.AxisListType
ALU = mybir.AluOpType
ACTF = mybir.ActivationFunctionType
DR = mybir.MatmulPerfMode.DoubleRow


def _body(tc, inp, cx, cy, cw_d, out_d, scale):
    nc = tc.nc
    from contextlib import ExitStack

    with ExitStack() as ctx:
        const = ctx.enter_context(tc.tile_pool(name="const", bufs=1))
        work = ctx.enter_context(tc.tile_pool(name="work", bufs=2))
        psum = ctx.enter_context(tc.tile_pool(name="psum", bufs=1, space="PSUM"))

        # ---- contiguous input DMAs ----------------------------------------
        A_nat = const.tile([128, NJ, DX], FP32)     # p = 4t + n//32, j = n%32
        inp_r = inp.rearrange("t (a j) d -> (t a) j d", a=4)
        nc.sync.dma_start(out=A_nat[:, 0:16, :], in_=inp_r[:, 0:16, :])
        nc.scalar.dma_start(out=A_nat[:, 16:32, :], in_=inp_r[:, 16:32, :])
        cx_nat = const.tile([128, MB, DX], FP32)    # m = 16p + r
        nc.sync.dma_start(out=cx_nat, in_=cx.rearrange("(p r) d -> p r d", p=128))
        cy_nat = const.tile([128, MB, DY], FP32)
        nc.gpsimd.dma_start(out=cy_nat, in_=cy.rearrange("(p r) d -> p r d", p=128))
        cw_nat = const.tile([128, MB], FP32)
        nc.gpsimd.dma_start(out=cw_nat, in_=cw_d.rearrange("(p r) -> p r", p=128))

        ident16 = const.tile([128, 128], FP16)
        make_identity(nc, ident16)
        ident32 = const.tile([DY + 1, DY + 1], FP32)
        make_identity(nc, ident32)

        # preload the exp table set during the prologue DMA wait
        warm = const.tile([128, 1], FP32)
        nc.gpsimd.memset(warm, 0.0)
        warm2 = const.tile([128, 1], FP32)
        nc.scalar.activation(warm2, warm, ACTF.Exp, bias=0.0, scale=1.0)

        # packed fp16 transpose sources
        cx_pack = const.tile([128, MB, KAUG], FP16)  # [d x32, 1.0, -b2/2]
        A_pack = const.tile([128, NJ, KAUG], FP16)   # [d x32, -a2/2, 1.0]
        nc.gpsimd.memset(cx_pack[:, :, DX : DX + 1], 1.0)
        nc.gpsimd.memset(A_pack[:, :, DX + 1 : DX + 2], 1.0)

        CT16 = const.tile([KAUG, MB, 128], FP16)
        AT16 = const.tile([KAUG, NJ, 128], FP16)
        W8 = const.tile([128, MB // 2, 2, 32], FP8)   # chunk-pair, padded to 32 cols
        nc.gpsimd.memset(W8, 0.0)
        S = psum.tile([32, 128], FP32, tag="S")
        K2r = const.tile([128, NRING, WIN], FP16)
        K2f = K2r.rearrange("p w f -> p (w f)")

        one3 = lambda t: t.rearrange("p (s o) -> p s o", o=1)

        # ---- prep helpers --------------------------------------------------
        def quant_chain(nat, pack, lo, hi, colh):
            sq = work.tile([128, NJ, DX], FP32, tag="sq", bufs=2)
            sqv = sq[:, 0 : hi - lo, :]
            nc.vector.tensor_mul(sqv, nat[:, lo:hi, :], nat[:, lo:hi, :])
            mh = work.tile([128, NJ], FP32, tag="mh", bufs=2)
            mhv = mh[:, 0 : hi - lo]
            nc.vector.tensor_reduce(out=one3(mhv), in_=sqv, axis=AX.X, op=ALU.add)
            nc.vector.tensor_scalar_mul(mhv, mhv, -0.5)
            nc.vector.tensor_copy(pack[:, lo:hi, colh : colh + 1], one3(mhv))
            nc.vector.tensor_copy(pack[:, lo:hi, 0:DX], nat[:, lo:hi, :])

        def transpose_one(pack, dst, idx, use_scalar):
            trk = psum.tile([KAUG, 128], FP16, tag="trk", bufs=3)
            nc.tensor.transpose(trk, pack[:, idx, :], ident16)
            if use_scalar:
                nc.scalar.copy(dst[:, idx, :], trk)
            else:
                nc.vector.tensor_copy(dst[:, idx, :], trk)

        def w_chain():
            sqy = work.tile([128, MB, DY], FP32, tag="sqy")
            nc.vector.tensor_mul(sqy, cy_nat, cy_nat)
            ssum = work.tile([128, MB], FP32, tag="ssum")
            nc.vector.tensor_reduce(out=one3(ssum), in_=sqy, axis=AX.X, op=ALU.add)
            rec = work.tile([128, MB], FP32, tag="rec")
            nc.vector.reciprocal(rec, ssum)
            facr = work.tile([128, MB], FP32, tag="facr")
            nc.vector.tensor_mul(facr, rec, cw_nat)
            facr_b = one3(facr).broadcast_to([128, MB, DY])
            wtmp = work.tile([128, MB, DY], FP32, tag="wtmp")
            nc.vector.tensor_mul(wtmp, sqy, facr_b)
            w8f = W8.rearrange("p a h c -> p (a h) c")     # [128, 16, 32]
            nc.vector.tensor_scalar_mul(w8f[:, :, 0:DY], wtmp, WSCL)
            nc.vector.tensor_scalar_mul(w8f[:, :, DY : DY + 1], one3(cw_nat), WSCL)

        # ---- prologue: cx chain, CT 0-1, AT block 0 -----------------------
        quant_chain(cx_nat, cx_pack, 0, MB, DX + 1)
        for c in range(2):
            transpose_one(cx_pack, CT16, c, True)
        quant_chain(A_nat, A_pack, 0, JB, DX)
        for j in range(JB):
            transpose_one(A_pack, AT16, j, True)

        # deferred prep, deadline-ordered: CT chunk c used at step c; AT
        # block b used from step 16b; W used by the first mm2 drain (~s8).
        def ct_q(c):
            return lambda: transpose_one(cx_pack, CT16, c, False)

        def at_q(j):
            return lambda: transpose_one(A_pack, AT16, j, False)

        def chain_q(jlo):
            return lambda: quant_chain(A_nat, A_pack, jlo, jlo + JB, DX)

        quanta = [ct_q(2), ct_q(3), ct_q(4), w_chain, ct_q(5), chain_q(JB)]
        for c in range(6, 14):
            quanta += [ct_q(c), at_q(c + 2)]     # AT block 1: j = 8..15
        quanta += [ct_q(14), ct_q(15)]
        for blk in (2, 3):
            jlo = blk * JB
            quanta.append(chain_q(jlo))
            quanta += [at_q(j) for j in range(jlo, jlo + JB)]
        # 2 quanta/step for the first 13 steps, then 1/step (done by s28).
        sched = {s: [] for s in range(64)}
        qi = 0
        for s in range(64):
            take = min(2 if s < 13 else 1, len(quanta) - qi)
            sched[s] = quanta[qi : qi + take]
            qi += take
        assert qi == len(quanta), (qi, len(quanta))

        AT16f = AT16.rearrange("p j c -> p (j c)")
        TOT = NBLK * MB

        # ---- main loop ----------------------------------------------------
        def emit_reduce(grp_base, nch):
            kv = K2f[:, grp_base : grp_base + nch * F_CHUNK].rearrange(
                "p (t f) -> p t f", t=nch
            )
            r1 = work.tile([128, GRP, 512], FP16, tag="r1")
            r1v = r1[:, 0:nch, :]
            nc.vector.tensor_add(r1v, kv[:, :, 0:512], kv[:, :, 512:1024])
            r2 = work.tile([128, GRP, 256], FP16, tag="r2")
            r2v = r2[:, 0:nch, :]
            nc.vector.tensor_add(r2v, r1v[:, :, 0:256], r1v[:, :, 256:512])
            r3 = work.tile([128, GRP, 128], FP16, tag="r3", bufs=2)
            r3v = r3[:, 0:nch, :]
            nc.vector.tensor_add(r3v, r2v[:, :, 0:128], r2v[:, :, 128:256])
            r38 = work.tile([128, GRP, 128], FP8, tag="r38", bufs=3)
            nc.vector.tensor_copy(r38[:, 0:nch, :], r3v)
            return r38

        def emit_mm2(entry):
            r38t, s0, nch = entry
            for i in range(0, nch, 2):
                s_ = s0 + i
                pr = (s_ % MB) // 2
                nc.tensor.matmul(
                    S,
                    W8[:, pr, :, :],
                    r38t[:, i : i + 2, :],
                    start=(s_ == 0),
                    stop=(s_ + 1 == TOT - 1),
                    perf_mode=DR,
                )
            return s0 + nch

        pending = []
        mm2_done = 0
        for s in range(TOT):
            jb, c = divmod(s, MB)
            gw = psum.tile([128, WIN], FP32, tag="g", bufs=2, name="gw")
            for q in range(2):
                nc.tensor.matmul(
                    gw[:, q * 512 : (q + 1) * 512],
                    CT16[:, c, :],
                    AT16f[:, jb * F_CHUNK + q * 512 : jb * F_CHUNK + (q + 1) * 512],
                    start=True,
                    stop=True,
                )
            nc.scalar.activation(
                K2r[:, s % NRING, :], gw, ACTF.Exp, bias=0.0, scale=scale
            )
            if s % GRP == GRP - 1 and s < GRP * (TOT // GRP):
                grp = s // GRP
                r38t = emit_reduce(WIN * ((GRP * grp) % NRING), GRP)
                pending.append((r38t, GRP * grp, GRP))
            while pending and pending[0][1] + pending[0][2] + 2 <= s:
                mm2_done = emit_mm2(pending.pop(0))
            for fn in sched[s]:
                fn()
        # tail: chunks 60..63 (ring slots 0..3)
        r38t = emit_reduce(WIN * (60 % NRING), 4)
        pending.append((r38t, 60, 4))
        for entry in pending:
            mm2_done = emit_mm2(entry)
        assert mm2_done == TOT

        # ---- epilogue: T = reduce(S), probs = T[:, :10] / T[:, 10] --------
        Tred = const.tile([DY + 1, BPC], FP32)
        nc.vector.tensor_reduce(
            out=Tred.rearrange("p (t o) -> p t o", o=1),
            in_=S[0 : DY + 1, :].rearrange("p (t f) -> p t f", f=4),
            axis=AX.X,
            op=ALU.add,
        )
        trT = psum.tile([BPC, DY + 1], FP32, tag="trk", bufs=3)
        nc.tensor.transpose(trT, Tred, ident32)
        Tt = const.tile([BPC, DY + 1], FP32)
        nc.vector.tensor_copy(Tt, trT)
        recd = const.tile([BPC, 1], FP32)
        nc.vector.reciprocal(recd, Tt[:, DY : DY + 1])
        outsb = const.tile([BPC, DY], FP32)
        nc.vector.tensor_scalar(
            out=outsb, in0=Tt[:, 0:DY], scalar1=recd, scalar2=None, op0=ALU.mult
        )
        nc.sync.dma_start(out=out_d, in_=outsb)


def build_program(scale):
    nc = bacc.Bacc(
        "TRN2",
        target_bir_lowering=False,
        debug=False,
        enable_asserts=False,
        num_devices=NCORES,
    )
    inp = nc.dram_tensor("inputs", [BPC, N, DX], FP32, kind="ExternalInput").ap()
    cx = nc.dram_tensor("c_x", [M, DX], FP32, kind="ExternalInput").ap()
    cy = nc.dram_tensor("c_y", [M, DY], FP32, kind="ExternalInput").ap()
    cw = nc.dram_tensor("comp_w", [M], FP32, kind="ExternalInput").ap()
    out = nc.dram_tensor("out", [BPC, DY], FP32, kind="ExternalOutput").ap()
    with tile.TileContext(nc) as tc:
        _body(tc, inp, cx, cy, cw, out, scale)
    nc.compile()
    return nc


_PROGRAM_CACHE: dict = {}


def _get_program(scale):
    nc = _PROGRAM_CACHE.get(scale)
    if nc is None:
        nc = build_program(scale)
        _PROGRAM_CACHE[scale] = nc
    return nc


def make_in_maps(inputs, c_x, c_y, comp_w):
    shards = np.ascontiguousarray(inputs.reshape(NCORES, BPC, N, DX))
    return [
        {
            "inputs": shards[i],
            "c_x": np.ascontiguousarray(c_x),
            "c_y": np.ascontiguousarray(c_y),
            "comp_w": np.ascontiguousarray(comp_w),
        }
        for i in range(NCORES)
    ]


def scale_from_sigma(sigma) -> float:
    s = max(float(np.asarray(sigma, dtype=np.float64)), MIN_SIGMA)
    return float(2.0 / (s * s))


def kernel(inputs, sigma, c_x, c_y, comp_w, _run_kwargs=None):
    nc = _get_program(scale_from_sigma(sigma))
    in_maps = make_in_maps(inputs, c_x, c_y, comp_w)
    res = run_bass_kernel_spmd(
        nc, in_maps, core_ids=list(range(NCORES)), **(_run_kwargs or {})
    )
    out = np.concatenate([res.results[i]["out"] for i in range(NCORES)], axis=0)
    return out.astype(np.float32)


# revision 17
# speedup vs baseline: 1.4021x; 1.0105x over previous
"""Trainium2 Bass kernel for nn_BagKQMClassModel.

Computation (per batch item b):
    K[b,n,m]   = exp(-d2/(2 s^2)),  d2 = |A[b,n] - C[m]|^2
    out_w[b,m] = (1/N) sum_n comp_w[m] * K^2
    y_w        = out_w / sum_m out_w
    probs      = y_w @ (y_v^2),  y_v = c_y rows normalized

Key transformations:
  * K^2 = exp(-d2/s^2): one exp per (b,n,m) element.
  * d2 folded into one fp16 matmul with augmented contraction (34 rows):
        rows 0-31: data a_d * c_d;  row 32: CT 1, AT -a2/2;
        row 33: CT -b2/2, AT 1
    so exp arg = (2/s^2) * g with no ACT bias.
  * probs = T[:, :10] / T[:, 10], T = sum_{n,m} K2 * W with
    W[m, :10] = comp_w[m] * c_y[m]^2 / |c_y[m]|^2, W[m, 10] = comp_w[m].
  * m and (b,n) are PERMUTED vs the reference layout so every input DMA
    is contiguous (partition p holds a contiguous DRAM block):
    m = 16p + r;  bn: p = 4t + n//32, j = n%32.  All downstream sums are
    permutation-invariant since c_y/comp_w use the same m-permutation.
  * One PSUM tile S[11, 128] accumulates mm2 over all 64 (m-chunk,
    j-block) steps; the final per-batch output is one [11, 32, 4]
    free-dim reduce + an 11x32 transpose.
  * mm2 runs as fp8e4 DoubleRow over CHUNK PAIRS (stationary W8
    [128, 2, 11], moving r3 [128, 2, 128]) - half the mm2 instruction
    count and half the stream cycles.  W8 is pre-scaled by 2048 so the
    tiny comp_w values stay inside fp8e4's dynamic range (min subnormal
    2^-9); the scale cancels in the final T[:, :10] / T[:, 10].
  * exp runs on 1024-wide ACT windows from a 2-deep PSUM ring (2 banks
    each; + 1 bank S + 3 x 1 bank transpose scratch = 8 banks).
  * The 8:1 bag reduction batches SIX chunks into one DVE op per
    halving level, amortizing the ~151-cycle DVE op overhead.
  * CT chunks 2-15, AT j-blocks 1-3, and the W build all run INSIDE the
    main loop in PE/DVE slack (deadline-scheduled quanta).

Sharding: batch 256 -> 32 items per core across 8 cores; c_x/c_y/comp_w
replicated. No collectives (forward only).
"""

import numpy as np

import concourse.bacc as bacc
import concourse.mybir as mybir
import concourse.tile as tile
from concourse.bass_utils import run_bass_kernel_spmd
from concourse.masks import make_identity

NCORES = 8
BS, N, DX, DY, M = 256, 128, 32, 10, 2048
BPC = BS // NCORES      # 32 batch items per core
MB = M // 128           # 16 chunks of the component axis
KAUG = DX + 2           # 34 augmented contraction rows
NJ = 32                 # bn = 32 j-slices x 128 p
JB = 8                  # j slices per block
NBLK = NJ // JB         # 4 blocks
F_CHUNK = JB * 128      # 1024 bn columns per (m-chunk, j-block) step
WIN = 1024              # ACT window = one chunk (2 x 512-col mm1 pieces)
NRING = 12              # K2 ring slots (6-chunk reduce groups never wrap)
GRP = 6                 # chunks per DVE reduce group
WSCL = 2048.0           # fp8 mm2 weight prescale (cancels in the division)
MIN_SIGMA = 1e-3
FP32 = mybir.dt.float32
FP16 = mybir.dt.float16
FP8 = mybir.dt.float8e4
AX = mybir.AxisListType
ALU = mybir.AluOpType
ACTF = mybir.ActivationFunctionType
DR = mybir.MatmulPerfMode.DoubleRow


def _body(tc, inp, cx, cy, cw_d, out_d, scale):
    nc = tc.nc
    from contextlib import ExitStack

    with ExitStack() as ctx:
        const = ctx.enter_context(tc.tile_pool(name="const", bufs=1))
        work = ctx.enter_context(tc.tile_pool(name="work", bufs=2))
        psum = ctx.enter_context(tc.tile_pool(name="psum", bufs=1, space="PSUM"))

        # ---- contiguous input DMAs ----------------------------------------
        A_nat = const.tile([128, NJ, DX], FP32)     # p = 4t + n//32, j = n%32
        inp_r = inp.rearrange("t (a j) d -> (t a) j d", a=4)
        nc.sync.dma_start(out=A_nat[:, 0:16, :], in_=inp_r[:, 0:16, :])
        nc.scalar.dma_start(out=A_nat[:, 16:32, :], in_=inp_r[:, 16:32, :])
        cx_nat = const.tile([128, MB, DX], FP32)    # m = 16p + r
        nc.sync.dma_start(out=cx_nat, in_=cx.rearrange("(p r) d -> p r d", p=128))
        cy_nat = const.tile([128, MB, DY], FP32)
        nc.gpsimd.dma_start(out=cy_nat, in_=cy.rearrange("(p r) d -> p r d", p=128))
        cw_nat = const.tile([128, MB], FP32)
        nc.gpsimd.dma_start(out=cw_nat, in_=cw_d.rearrange("(p r) -> p r", p=128))

        ident16 = const.tile([128, 128], FP16)
        make_identity(nc, ident16)
        ident32 = const.tile([DY + 1, DY + 1], FP32)
        make_identity(nc, ident32)

        # preload the exp table set during the prologue DMA wait
        warm = const.tile([128, 1], FP32)
        nc.gpsimd.memset(warm, 0.0)
        warm2 = const.tile([128, 1], FP32)
        nc.scalar.activation(warm2, warm, ACTF.Exp, bias=0.0, scale=1.0)

        # packed fp16 transpose sources
        cx_pack = const.tile([128, MB, KAUG], FP16)  # [d x32, 1.0, -b2/2]
        A_pack = const.tile([128, NJ, KAUG], FP16)   # [d x32, -a2/2, 1.0]
        nc.gpsimd.memset(cx_pack[:, :, DX : DX + 1], 1.0)
        nc.gpsimd.memset(A_pack[:, :, DX + 1 : DX + 2], 1.0)

        CT16 = const.tile([KAUG, MB, 128], FP16)
        AT16 = const.tile([KAUG, NJ, 128], FP16)
        W8 = const.tile([128, MB // 2, 2, 32], FP8)   # chunk-pair, padded to 32 cols
        nc.gpsimd.memset(W8, 0.0)
        S = psum.tile([32, 128], FP32, tag="S")
        K2r = const.tile([128, NRING, WIN], FP16)
        K2f = K2r.rearrange("p w f -> p (w f)")

        one3 = lambda t: t.rearrange("p (s o) -> p s o", o=1)

        # ---- prep helpers --------------------------------------------------
        def quant_chain(nat, pack, lo, hi, colh):
            sq = work.tile([128, NJ, DX], FP32, tag="sq", bufs=2)
            sqv = sq[:, 0 : hi - lo, :]
            nc.vector.tensor_mul(sqv, nat[:, lo:hi, :], nat[:, lo:hi, :])
            mh = work.tile([128, NJ], FP32, tag="mh", bufs=2)
            mhv = mh[:, 0 : hi - lo]
            nc.vector.tensor_reduce(out=one3(mhv), in_=sqv, axis=AX.X, op=ALU.add)
            nc.vector.tensor_scalar_mul(mhv, mhv, -0.5)
            nc.vector.tensor_copy(pack[:, lo:hi, colh : colh + 1], one3(mhv))
            nc.vector.tensor_copy(pack[:, lo:hi, 0:DX], nat[:, lo:hi, :])

        def transpose_one(pack, dst, idx, use_scalar):
            trk = psum.tile([KAUG, 128], FP16, tag="trk", bufs=3)
            nc.tensor.transpose(trk, pack[:, idx, :], ident16)
            if use_scalar:
                nc.scalar.copy(dst[:, idx, :], trk)
            else:
                nc.vector.tensor_copy(dst[:, idx, :], trk)

        def w_chain():
            sqy = work.tile([128, MB, DY], FP32, tag="sqy")
            nc.vector.tensor_mul(sqy, cy_nat, cy_nat)
            ssum = work.tile([128, MB], FP32, tag="ssum")
            nc.vector.tensor_reduce(out=one3(ssum), in_=sqy, axis=AX.X, op=ALU.add)
            rec = work.tile([128, MB], FP32, tag="rec")
            nc.vector.reciprocal(rec, ssum)
            facr = work.tile([128, MB], FP32, tag="facr")
            nc.vector.tensor_mul(facr, rec, cw_nat)
            facr_b = one3(facr).broadcast_to([128, MB, DY])
            wtmp = work.tile([128, MB, DY], FP32, tag="wtmp")
            nc.vector.tensor_mul(wtmp, sqy, facr_b)
            w8f = W8.rearrange("p a h c -> p (a h) c")     # [128, 16, 32]
            nc.vector.tensor_scalar_mul(w8f[:, :, 0:DY], wtmp, WSCL)
            nc.vector.tensor_scalar_mul(w8f[:, :, DY : DY + 1], one3(cw_nat), WSCL)

        # ---- prologue: cx chain, CT 0-1, AT block 0 -----------------------
        quant_chain(cx_nat, cx_pack, 0, MB, DX + 1)
        for c in range(2):
            transpose_one(cx_pack, CT16, c, True)
        quant_chain(A_nat, A_pack, 0, JB, DX)
        for j in range(JB):
            transpose_one(A_pack, AT16, j, True)

        # deferred prep, deadline-ordered: CT chunk c used at step c; AT
        # block b used from step 16b; W used by the first mm2 drain (~s8).
        def ct_q(c):
            return lambda: transpose_one(cx_pack, CT16, c, False)

        def at_q(j):
            return lambda: transpose_one(A_pack, AT16, j, False)

        def chain_q(jlo):
            return lambda: quant_chain(A_nat, A_pack, jlo, jlo + JB, DX)

        quanta = [ct_q(2), ct_q(3), ct_q(4), w_chain, ct_q(5), chain_q(JB)]
        for c in range(6, 14):
            quanta += [ct_q(c), at_q(c + 2)]     # AT block 1: j = 8..15
        quanta += [ct_q(14), ct_q(15)]
        for blk in (2, 3):
            jlo = blk * JB
            quanta.append(chain_q(jlo))
            quanta += [at_q(j) for j in range(jlo, jlo + JB)]
        # 2 quanta/step for the first 13 steps, then 1/step (done by s28).
        sched = {s: [] for s in range(64)}
        qi = 0
        for s in range(64):
            take = min(2 if s < 13 else 1, len(quanta) - qi)
            sched[s] = quanta[qi : qi + take]
            qi += take
        assert qi == len(quanta), (qi, len(quanta))

        AT16f = AT16.rearrange("p j c -> p (j c)")
        TOT = NBLK * MB

        # ---- main loop ----------------------------------------------------
        def emit_reduce(grp_base, nch):
            kv = K2f[:, grp_base : grp_base + nch * F_CHUNK].rearrange(
                "p (t f) -> p t f", t=nch
            )
            r1 = work.tile([128, GRP, 512], FP16, tag="r1")
            r1v = r1[:, 0:nch, :]
            nc.vector.tensor_add(r1v, kv[:, :, 0:512], kv[:, :, 512:1024])
            r2 = work.tile([128, GRP, 256], FP16, tag="r2")
            r2v = r2[:, 0:nch, :]
            nc.vector.tensor_add(r2v, r1v[:, :, 0:256], r1v[:, :, 256:512])
            r3 = work.tile([128, GRP, 128], FP16, tag="r3", bufs=2)
            r3v = r3[:, 0:nch, :]
            nc.vector.tensor_add(r3v, r2v[:, :, 0:128], r2v[:, :, 128:256])
            r38 = work.tile([128, GRP, 128], FP8, tag="r38", bufs=3)
            nc.vector.tensor_copy(r38[:, 0:nch, :], r3v)
            return r38

        def emit_mm2(entry):
            r38t, s0, nch = entry
            for i in range(0, nch, 2):
                s_ = s0 + i
                pr = (s_ % MB) // 2
                nc.tensor.matmul(
                    S,
                    W8[:, pr, :, :],
                    r38t[:, i : i + 2, :],
                    start=(s_ == 0),
                    stop=(s_ + 1 == TOT - 1),
                    perf_mode=DR,
                )
            return s0 + nch

        # p-state pre-warm: ~5us of back-to-back matmuls right before the
        # loop so the PE enters the loop at its ramped clock (measured:
        # ramp engages after ~10 gapless matmuls).
        warm_mov = ident16.rearrange("p (o c) -> p o c", o=1).broadcast_to(
            [128, 4, 128]
        )
        for _ in range(12):
            gwu = psum.tile([128, WIN], FP32, tag="g", bufs=2, name="gwu")
            nc.tensor.matmul(gwu[:, 0:512], ident16, warm_mov, start=True, stop=True)

        pending = []
        mm2_done = 0
        for s in range(TOT):
            jb, c = divmod(s, MB)
            gw = psum.tile([128, WIN], FP32, tag="g", bufs=2, name="gw")
            for q in range(2):
                nc.tensor.matmul(
                    gw[:, q * 512 : (q + 1) * 512],
                    CT16[:, c, :],
                    AT16f[:, jb * F_CHUNK + q * 512 : jb * F_CHUNK + (q + 1) * 512],
                    start=True,
                    stop=True,
                )
            nc.scalar.activation(
                K2r[:, s % NRING, :], gw, ACTF.Exp, bias=0.0, scale=scale
            )
            if s % GRP == GRP - 1 and s < GRP * (TOT // GRP):
                grp = s // GRP
                r38t = emit_reduce(WIN * ((GRP * grp) % NRING), GRP)
                pending.append((r38t, GRP * grp, GRP))
            while pending and pending[0][1] + pending[0][2] + 2 <= s:
                mm2_done = emit_mm2(pending.pop(0))
            for fn in sched[s]:
                fn()
        # tail: chunks 60..63 (ring slots 0..3)
        r38t = emit_reduce(WIN * (60 % NRING), 4)
        pending.append((r38t, 60, 4))
        for entry in pending:
            mm2_done = emit_mm2(entry)
        assert mm2_done == TOT

        # ---- epilogue: T = reduce(S), probs = T[:, :10] / T[:, 10] --------
        Tred = const.tile([DY + 1, BPC], FP32)
        nc.vector.tensor_reduce(
            out=Tred.rearrange("p (t o) -> p t o", o=1),
            in_=S[0 : DY + 1, :].rearrange("p (t f) -> p t f", f=4),
            axis=AX.X,
            op=ALU.add,
        )
        trT = psum.tile([BPC, DY + 1], FP32, tag="trk", bufs=3)
        nc.tensor.transpose(trT, Tred, ident32)
        Tt = const.tile([BPC, DY + 1], FP32)
        nc.vector.tensor_copy(Tt, trT)
        recd = const.tile([BPC, 1], FP32)
        nc.vector.reciprocal(recd, Tt[:, DY : DY + 1])
        outsb = const.tile([BPC, DY], FP32)
        nc.vector.tensor_scalar(
            out=outsb, in0=Tt[:, 0:DY], scalar1=recd, scalar2=None, op0=ALU.mult
        )
        nc.sync.dma_start(out=out_d, in_=outsb)


def build_program(scale):
    nc = bacc.Bacc(
        "TRN2",
        target_bir_lowering=False,
        debug=False,
        enable_asserts=False,
        num_devices=NCORES,
    )
    inp = nc.dram_tensor("inputs", [BPC, N, DX], FP32, kind="ExternalInput").ap()
    cx = nc.dram_tensor("c_x", [M, DX], FP32, kind="ExternalInput").ap()
    cy = nc.dram_tensor("c_y", [M, DY], FP32, kind="ExternalInput").ap()
    cw = nc.dram_tensor("comp_w", [M], FP32, kind="ExternalInput").ap()
    out = nc.dram_tensor("out", [BPC, DY], FP32, kind="ExternalOutput").ap()
    with tile.TileContext(nc) as tc:
        _body(tc, inp, cx, cy, cw, out, scale)
    nc.compile()
    return nc


_PROGRAM_CACHE: dict = {}


def _get_program(scale):
    nc = _PROGRAM_CACHE.get(scale)
    if nc is None:
        nc = build_program(scale)
        _PROGRAM_CACHE[scale] = nc
    return nc


def make_in_maps(inputs, c_x, c_y, comp_w):
    shards = np.ascontiguousarray(inputs.reshape(NCORES, BPC, N, DX))
    return [
        {
            "inputs": shards[i],
            "c_x": np.ascontiguousarray(c_x),
            "c_y": np.ascontiguousarray(c_y),
            "comp_w": np.ascontiguousarray(comp_w),
        }
        for i in range(NCORES)
    ]


def scale_from_sigma(sigma) -> float:
    s = max(float(np.asarray(sigma, dtype=np.float64)), MIN_SIGMA)
    return float(2.0 / (s * s))


def kernel(inputs, sigma, c_x, c_y, comp_w, _run_kwargs=None):
    nc = _get_program(scale_from_sigma(sigma))
    in_maps = make_in_maps(inputs, c_x, c_y, comp_w)
    res = run_bass_kernel_spmd(
        nc, in_maps, core_ids=list(range(NCORES)), **(_run_kwargs or {})
    )
    out = np.concatenate([res.results[i]["out"] for i in range(NCORES)], axis=0)
    return out.astype(np.float32)
